# revision 17
# baseline (speedup 1.0000x reference)
"""Trainium2 Bass kernel for degree-3 real spherical-harmonics evaluation.

Computes, for N=2M points with 16 SH coefficients x 2 channels each:
    d    = normalize(coordinates - rx_pos)
    out  = sum_k basis_k(d) * sh[n, k, c]

Strategy (8 NeuronCores, data-parallel over points):
  - Pad N to 2,007,040 = 8 cores * 128 partitions * 1960 points and give each
    core a contiguous slab. Per core, point n lives at (partition p = n//1960,
    f = n%1960); all DMAs are large and fully contiguous per partition.
  - sh coefficients are DMA'd with an inline fp32->bf16 cast (SWDGE), then one
    ScalarE copy de-interleaves them into per-(k,c) planes so the vector MAC
    runs in bf16 2x perf mode with unit stride.
  - The SH basis is built from C1-scaled unit vectors; every SH constant is
    folded into fused DVE ops (scalar_tensor_tensor / affine_mul_reduce /
    tensor_scalar), so no separate scale passes are needed.
  - MAC: 15 broadcasted bf16 multiplies (both channels per instruction) and a
    16-term binary add tree, all in 2x mode.
"""

import ml_dtypes
import numpy as np

import concourse.bass as bass
import concourse.tile as tile
from concourse import bacc, mybir
from concourse.bass_utils import run_bass_kernel_spmd

f32 = mybir.dt.float32
bf16 = mybir.dt.bfloat16
AF = mybir.ActivationFunctionType
OP = mybir.AluOpType

# ----- problem constants (hardcoded per spec) -----
N = 2_000_000
K = 16
CH = 2
ACTIVE_DEG = 3

C0 = 0.28209479177387814
C1 = 0.4886025119029199
C2 = (1.0925484305920792, -1.0925484305920792, 0.31539156525252005,
      -1.0925484305920792, 0.5462742152960396)
C3 = (-0.5900435899266435, 2.890611442640554, -0.4570457994644658,
      0.3731763325901154, -0.4570457994644658, 1.445305721320277,
      -0.5900435899266435)

# Basis constants with the C1 hat-scaling folded in (hats carry a factor C1).
_C12 = C1 * C1
_C13 = C1 * C1 * C1
CC4 = C2[0] / _C12
CC5 = C2[1] / _C12
A6, D6 = 3.0 * C2[2] / _C12, -C2[2]
CC7 = C2[3] / _C12
CC8 = C2[4] / _C12
CC9 = C3[0] / _C13
CC10 = C3[1] / _C13
A11, D11 = 5.0 * C3[2] / _C13, -C3[2] / C1
A12, D12 = 5.0 * C3[3] / _C13, -3.0 * C3[3] / C1
A13, D13 = 5.0 * C3[4] / _C13, -C3[4] / C1
CC14 = C3[5] / _C13
CC15 = C3[6] / _C13

# ----- sharding geometry -----
NCORES = 8
PPART = 1960                 # points per partition per core
PC = 128 * PPART             # points per core = 250,880
NPAD = NCORES * PC           # 2,007,040
TF = 392                     # points per partition per tile
NT = PPART // TF             # 5 tiles


def _build_nc():
    # Inputs arrive host-preprocessed: coords as 3 planes [3, PC] fp32 and
    # sh as 32 (k,c)-planes [32, PC] bf16 — every DMA lands in the exact
    # SBUF layout compute wants, no on-chip shuffling.
    nc = bacc.Bacc("TRN2")
    coords_ext = nc.declare_dram_parameter("coords", [3, PC], f32, isOutput=False)
    sh_ext = nc.declare_dram_parameter("sh", [32, PC], bf16, isOutput=False)
    consts_ext = nc.declare_dram_parameter("consts", [128, 4], f32, isOutput=False)
    out_ext = nc.declare_dram_parameter("out", [PC, CH], f32, isOutput=True)

    coords_ap = coords_ext[:].rearrange("c (p f) -> p c f", p=128)   # [128,3,1960]
    sh_ap = sh_ext[:].rearrange("j (p f) -> p j f", p=128)           # [128,32,1960]
    out_ap = out_ext[:].rearrange("(p f) c -> p (f c)", p=128)       # [128, 3920]

    F = TF
    with tile.TileContext(nc) as tc:
        with (
            tc.tile_pool(name="pconst", bufs=1) as pconst,
            tc.tile_pool(name="psh", bufs=2) as psh,
            tc.tile_pool(name="pco", bufs=2) as pco,
            tc.tile_pool(name="psq", bufs=2) as psq,
            tc.tile_pool(name="pr", bufs=2) as pr,
            tc.tile_pool(name="ph", bufs=2) as ph,
            tc.tile_pool(name="pmono", bufs=2) as pmono,
            tc.tile_pool(name="pb", bufs=3) as pb,
            tc.tile_pool(name="pm", bufs=3) as pm,
            tc.tile_pool(name="ptree", bufs=4) as ptree,
            tc.tile_pool(name="pacc", bufs=2) as pacc,
            tc.tile_pool(name="pout", bufs=2) as pout,
            tc.tile_pool(name="pscr", bufs=2) as pscr,
        ):
            ct = pconst.tile([128, 4], f32)
            nc.sync.dma_start(out=ct[:], in_=consts_ext[:])

            for t in range(NT):
                shin = psh.tile([128, F * 32], bf16, tag="shin")
                nc.sync.dma_start(
                    out=shin[:].rearrange("p (j f) -> p j f", f=F),
                    in_=sh_ap[:, :, t * F:(t + 1) * F],
                )
                ctile = pco.tile([128, F * 3], f32, tag="ctile")
                nc.sync.dma_start(
                    out=ctile[:].rearrange("p (c f) -> p c f", f=F),
                    in_=coords_ap[:, :, t * F:(t + 1) * F],
                )

                cv = ctile[:].rearrange("p (c f) -> p c f", c=3)  # plane comps

                # squared offsets (x-rx)^2 via Square's free affine
                sq = psq.tile([128, 3 * F], f32, tag="sq")
                for i in range(3):
                    nc.scalar.activation(
                        sq[:, i * F:(i + 1) * F], cv[:, i, :], AF.Square,
                        bias=ct[:, i:i + 1], scale=1.0,
                    )

                r2a = pr.tile([128, F], f32, tag="r2a")
                nc.vector.tensor_add(r2a[:], sq[:, 0:F], sq[:, F:2 * F])
                nc.vector.scalar_tensor_tensor(
                    r2a[:], sq[:, 2 * F:3 * F], 1e-12, r2a[:], OP.add, OP.add
                )
                inv = pr.tile([128, F], f32, tag="inv")
                nc.vector.reciprocal_approx_fast(inv[:], r2a[:])
                # sqrt(C1^2 / r2) = C1 * rsqrt(r2)
                rinv = inv
                nc.scalar.activation(rinv[:], inv[:], AF.Sqrt, bias=0.0,
                                     scale=_C12)

                # C1-scaled unit vector: ((x + (-rx)) * rinvC1), fp32
                hats = ph.tile([128, 3 * F], f32, tag="hats")
                for i in range(3):
                    nc.vector.scalar_tensor_tensor(
                        hats[:, i * F:(i + 1) * F], cv[:, i, :], ct[:, i:i + 1],
                        rinv[:], OP.add, OP.mult,
                    )
                X = hats[:, 0:F]
                Y = hats[:, F:2 * F]
                Z = hats[:, 2 * F:3 * F]

                sqh = ph.tile([128, 3 * F], f32, tag="sqh")
                nc.scalar.activation(sqh[:], hats[:], AF.Square, bias=0.0,
                                     scale=1.0)
                XX = sqh[:, 0:F]
                YY = sqh[:, F:2 * F]
                ZZ = sqh[:, 2 * F:3 * F]

                xy = pmono.tile([128, F], f32, tag="xy")
                nc.vector.tensor_mul(xy[:], X, Y)
                t8 = pmono.tile([128, F], f32, tag="t8")
                nc.vector.tensor_sub(t8[:], XX, YY)
                u9 = pmono.tile([128, F], f32, tag="u9")
                nc.vector.affine_then_add(u9[:], XX, t8[:], 2.0, 0.0)
                u15 = pmono.tile([128, F], f32, tag="u15")
                nc.vector.affine_then_add(u15[:], YY, t8[:], -2.0, 0.0)

                # ---- MAC: pair-batched products ([k2, c, f] = 4F per op) ----
                # Basis planes are packed in pairs matching consecutive k so
                # each product instruction covers 2 k's x 2 channels, and the
                # add tree runs on [4F] chunks (terms halve each level).
                def bpair_tile():
                    return pb.tile([128, 2 * F], bf16, tag="b", name="bp")

                def mk_product2(p_idx, bp):
                    m = pm.tile([128, 4 * F], bf16, tag="m", name="m")
                    in1 = shin[:, 4 * p_idx * F:(4 * p_idx + 4) * F].rearrange(
                        "p (k c f) -> p k c f", k=2, c=2)
                    in0 = bp[:].rearrange("p (k f) -> p k f", k=2) \
                        .unsqueeze(2).broadcast_to((128, 2, 2, F))
                    nc.vector.tensor_tensor(
                        m[:].rearrange("p (k c f) -> p k c f", k=2, c=2),
                        in0, in1, OP.mult)
                    return m

                def amr(out_slice, in0, in1, scale, bias):
                    scr = pscr.tile([128, 1], f32, tag="scr", name="scr")
                    nc.vector.affine_mul_reduce(out_slice, scr[:], in0, in1,
                                                scale, bias)

                def tadd(a, b_, dt):
                    tg = "treeb" if dt == bf16 else "treef"
                    nb = 4
                    o = ptree.tile([128, a.shape[1]], dt, tag=tg, name="tr",
                                   bufs=nb)
                    nc.vector.tensor_tensor(o[:], a[:], b_[:], OP.add)
                    return o

                # pair (k0, k1): [C0 const, -Ytilde]
                bp0 = bpair_tile()
                nc.vector.memset(bp0[:, 0:F], C0)
                nc.vector.tensor_scalar_mul(bp0[:, F:2 * F], Y, -1.0)
                m0 = mk_product2(0, bp0)
                # pair (k2, k3): [Ztilde, -Xtilde]
                bp1 = bpair_tile()
                nc.vector.tensor_scalar_mul(bp1[:, 0:F], Z, 1.0)
                nc.vector.tensor_scalar_mul(bp1[:, F:2 * F], X, -1.0)
                m1 = mk_product2(1, bp1)
                A = tadd(m0, m1, bf16)

                # pair (k4, k5): [c4*xy, c5*Y*Z]
                bp2 = bpair_tile()
                nc.vector.tensor_scalar_mul(bp2[:, 0:F], xy[:], CC4)
                nc.vector.scalar_tensor_tensor(bp2[:, F:2 * F], Y, CC5, Z,
                                               OP.mult, OP.mult)
                m2 = mk_product2(2, bp2)
                # pair (k6, k7): [a6*ZZ+d6, c7*X*Z]
                bp3 = bpair_tile()
                nc.vector.tensor_scalar(bp3[:, 0:F], ZZ, A6, D6, OP.mult, OP.add)
                nc.vector.scalar_tensor_tensor(bp3[:, F:2 * F], X, CC7, Z,
                                               OP.mult, OP.mult)
                m3 = mk_product2(3, bp3)
                B = tadd(m2, m3, bf16)

                # pair (k8, k9): [c8*t8, c9*u9*Y]
                bp4 = bpair_tile()
                nc.vector.tensor_scalar_mul(bp4[:, 0:F], t8[:], CC8)
                amr(bp4[:, F:2 * F], u9[:], Y, CC9, 0.0)
                m4 = mk_product2(4, bp4)
                # pair (k10, k11): [c10*xy*Z, (a11*ZZ+d11)*Y]
                bp5 = bpair_tile()
                amr(bp5[:, 0:F], xy[:], Z, CC10, 0.0)
                amr(bp5[:, F:2 * F], ZZ, Y, A11, D11)
                m5 = mk_product2(5, bp5)
                Cc = tadd(m4, m5, bf16)

                # pair (k12, k13)
                bp6 = bpair_tile()
                amr(bp6[:, 0:F], ZZ, Z, A12, D12)
                amr(bp6[:, F:2 * F], ZZ, X, A13, D13)
                m6 = mk_product2(6, bp6)
                # pair (k14, k15)
                bp7 = bpair_tile()
                amr(bp7[:, 0:F], t8[:], Z, CC14, 0.0)
                amr(bp7[:, F:2 * F], u15[:], X, CC15, 0.0)
                m7 = mk_product2(7, bp7)
                D = tadd(m6, m7, bf16)

                # upper tree levels on the (otherwise idle) GpSimd engine
                def gadd(a, b_):
                    o = ptree.tile([128, a.shape[1]], f32, tag="treef",
                                   name="tr", bufs=4)
                    nc.gpsimd.tensor_add(o[:], a[:], b_[:])
                    return o

                E = gadd(A, B)
                G = gadd(Cc, D)
                H = gadd(E, G)
                acc = pacc.tile([128, 2 * F], f32, tag="acc")
                nc.gpsimd.tensor_add(acc[:], H[:, 0:2 * F], H[:, 2 * F:4 * F])

                # interleave back to (f, c) and upcast to fp32
                out_t = pout.tile([128, 2 * F], f32, tag="out")
                nc.scalar.copy(
                    out_t[:].rearrange("p (f c) -> p c f", c=2),
                    acc[:].rearrange("p (c f) -> p c f", c=2),
                )
                nc.sync.dma_start(
                    out=out_ap[:, t * 2 * F:(t + 1) * 2 * F], in_=out_t[:]
                )

    nc.finalize()
    return nc


_NC_CACHE = None
_last_in_maps = None


def _get_nc():
    global _NC_CACHE
    if _NC_CACHE is None:
        _NC_CACHE = _build_nc()
    return _NC_CACHE


def kernel(coordinates, active_deg, max_coeffs, sh_coefficients, rx_pos,
           **unused):
    assert int(active_deg) == ACTIVE_DEG and int(max_coeffs) == K
    coords = np.ascontiguousarray(np.asarray(coordinates, dtype=np.float32))
    sh = np.ascontiguousarray(np.asarray(sh_coefficients, dtype=np.float32))
    rx = np.asarray(rx_pos, dtype=np.float32).reshape(3)
    n = coords.shape[0]
    assert n == N and sh.shape == (N * K, CH)

    consts = np.zeros((128, 4), dtype=np.float32)
    consts[:, 0:3] = -rx[None, :]

    # Host-side relayout: coords -> 3 fp32 planes, sh -> 32 bf16 (k,c)-planes,
    # so device DMAs land directly in compute layout.
    sh32 = sh.reshape(n, K * CH)
    coordsT = coords.T  # [3, N] view
    in_maps = []
    for c in range(NCORES):
        lo, hi = c * PC, (c + 1) * PC
        real = min(hi, n) - lo
        coords_c = np.zeros((3, PC), dtype=np.float32)
        coords_c[:, :real] = coordsT[:, lo:lo + real]
        sh_c = np.zeros((32, PC), dtype=ml_dtypes.bfloat16)
        sh_c[:, :real] = sh32[lo:lo + real].T
        in_maps.append({"coords": coords_c, "sh": sh_c, "consts": consts})

    global _last_in_maps
    _last_in_maps = in_maps
    res = run_bass_kernel_spmd(_get_nc(), in_maps, list(range(NCORES)))
    out = np.concatenate([np.asarray(res.results[c]["out"])
                          for c in range(NCORES)], axis=0)
    return out[:n]


# revision 19
# speedup vs baseline: 1.1891x; 1.1891x over previous
"""Trainium2 Bass kernel for degree-3 real spherical-harmonics evaluation.

Computes, for N=2M points with 16 SH coefficients x 2 channels each:
    d    = normalize(coordinates - rx_pos)
    out  = sum_k basis_k(d) * sh[n, k, c]

Strategy (8 NeuronCores, data-parallel over points):
  - Pad N to 2,007,040 = 8 cores * 128 partitions * 1960 points and give each
    core a contiguous slab. Per core, point n lives at (partition p = n//1960,
    f = n%1960); all DMAs are large and fully contiguous per partition.
  - sh coefficients are DMA'd with an inline fp32->bf16 cast (SWDGE), then one
    ScalarE copy de-interleaves them into per-(k,c) planes so the vector MAC
    runs in bf16 2x perf mode with unit stride.
  - The SH basis is built from C1-scaled unit vectors; every SH constant is
    folded into fused DVE ops (scalar_tensor_tensor / affine_mul_reduce /
    tensor_scalar), so no separate scale passes are needed.
  - MAC: 15 broadcasted bf16 multiplies (both channels per instruction) and a
    16-term binary add tree, all in 2x mode.
"""

import ml_dtypes
import numpy as np

import concourse.bass as bass
import concourse.tile as tile
from concourse import bacc, mybir
from concourse.bass_utils import run_bass_kernel_spmd

f32 = mybir.dt.float32
bf16 = mybir.dt.bfloat16
AF = mybir.ActivationFunctionType
OP = mybir.AluOpType

# ----- problem constants (hardcoded per spec) -----
N = 2_000_000
K = 16
CH = 2
ACTIVE_DEG = 3

C0 = 0.28209479177387814
C1 = 0.4886025119029199
C2 = (1.0925484305920792, -1.0925484305920792, 0.31539156525252005,
      -1.0925484305920792, 0.5462742152960396)
C3 = (-0.5900435899266435, 2.890611442640554, -0.4570457994644658,
      0.3731763325901154, -0.4570457994644658, 1.445305721320277,
      -0.5900435899266435)

# Basis constants with the C1 hat-scaling folded in (hats carry a factor C1).
_C12 = C1 * C1
_C13 = C1 * C1 * C1
CC4 = C2[0] / _C12
CC5 = C2[1] / _C12
A6, D6 = 3.0 * C2[2] / _C12, -C2[2]
CC7 = C2[3] / _C12
CC8 = C2[4] / _C12
CC9 = C3[0] / _C13
CC10 = C3[1] / _C13
A11, D11 = 5.0 * C3[2] / _C13, -C3[2] / C1
A12, D12 = 5.0 * C3[3] / _C13, -3.0 * C3[3] / C1
A13, D13 = 5.0 * C3[4] / _C13, -C3[4] / C1
CC14 = C3[5] / _C13
CC15 = C3[6] / _C13

# ----- sharding geometry -----
NCORES = 8
PPART = 1960                 # points per partition per core
PC = 128 * PPART             # points per core = 250,880
NPAD = NCORES * PC           # 2,007,040
TF = 392                     # points per partition per tile
NT = PPART // TF             # 5 tiles


def _build_nc():
    # Inputs arrive host-preprocessed: coords as 3 planes [3, PC] fp32 and
    # sh as 32 (k,c)-planes [32, PC] bf16 — every DMA lands in the exact
    # SBUF layout compute wants, no on-chip shuffling.
    nc = bacc.Bacc("TRN2")
    coords_ext = nc.declare_dram_parameter("coords", [3, PC], f32, isOutput=False)
    sh_ext = nc.declare_dram_parameter("sh", [32, PC], bf16, isOutput=False)
    consts_ext = nc.declare_dram_parameter("consts", [128, 4], f32, isOutput=False)
    out_ext = nc.declare_dram_parameter("out", [PC, CH], f32, isOutput=True)

    coords_ap = coords_ext[:].rearrange("c (p f) -> p c f", p=128)   # [128,3,1960]
    sh_ap = sh_ext[:].rearrange("j (p f) -> p j f", p=128)           # [128,32,1960]
    out_ap = out_ext[:].rearrange("(p f) c -> p (f c)", p=128)       # [128, 3920]

    F = TF
    with tile.TileContext(nc) as tc:
        with (
            tc.tile_pool(name="pconst", bufs=1) as pconst,
            tc.tile_pool(name="psh", bufs=2) as psh,
            tc.tile_pool(name="pco", bufs=2) as pco,
            tc.tile_pool(name="psq", bufs=2) as psq,
            tc.tile_pool(name="pr", bufs=2) as pr,
            tc.tile_pool(name="ph", bufs=2) as ph,
            tc.tile_pool(name="pmono", bufs=2) as pmono,
            tc.tile_pool(name="pb", bufs=3) as pb,
            tc.tile_pool(name="pm", bufs=3) as pm,
            tc.tile_pool(name="ptree", bufs=4) as ptree,
            tc.tile_pool(name="pacc", bufs=2) as pacc,
            tc.tile_pool(name="pout", bufs=2) as pout,
            tc.tile_pool(name="pscr", bufs=2) as pscr,
        ):
            ct = pconst.tile([128, 4], f32)
            nc.sync.dma_start(out=ct[:], in_=consts_ext[:])

            for t in range(NT):
                shin = psh.tile([128, F * 32], bf16, tag="shin")
                nc.sync.dma_start(
                    out=shin[:].rearrange("p (j f) -> p j f", f=F),
                    in_=sh_ap[:, :, t * F:(t + 1) * F],
                )
                ctile = pco.tile([128, F * 3], f32, tag="ctile")
                nc.sync.dma_start(
                    out=ctile[:].rearrange("p (c f) -> p c f", f=F),
                    in_=coords_ap[:, :, t * F:(t + 1) * F],
                )

                cv = ctile[:].rearrange("p (c f) -> p c f", c=3)  # plane comps

                # squared offsets (x-rx)^2 via Square's free affine
                sq = psq.tile([128, 3 * F], f32, tag="sq")
                for i in range(3):
                    nc.scalar.activation(
                        sq[:, i * F:(i + 1) * F], cv[:, i, :], AF.Square,
                        bias=ct[:, i:i + 1], scale=1.0,
                    )

                r2a = pr.tile([128, F], f32, tag="r2a")
                nc.vector.tensor_add(r2a[:], sq[:, 0:F], sq[:, F:2 * F])
                nc.vector.scalar_tensor_tensor(
                    r2a[:], sq[:, 2 * F:3 * F], 1e-12, r2a[:], OP.add, OP.add
                )
                inv = pr.tile([128, F], f32, tag="inv")
                nc.vector.reciprocal_approx_fast(inv[:], r2a[:])
                # sqrt(C1^2 / r2) = C1 * rsqrt(r2)
                rinv = inv
                nc.scalar.activation(rinv[:], inv[:], AF.Sqrt, bias=0.0,
                                     scale=_C12)

                # C1-scaled unit vector: ((x + (-rx)) * rinvC1), fp32
                hats = ph.tile([128, 3 * F], f32, tag="hats")
                for i in range(3):
                    nc.vector.scalar_tensor_tensor(
                        hats[:, i * F:(i + 1) * F], cv[:, i, :], ct[:, i:i + 1],
                        rinv[:], OP.add, OP.mult,
                    )
                X = hats[:, 0:F]
                Y = hats[:, F:2 * F]
                Z = hats[:, 2 * F:3 * F]

                sqh = ph.tile([128, 3 * F], f32, tag="sqh")
                nc.scalar.activation(sqh[:], hats[:], AF.Square, bias=0.0,
                                     scale=1.0)
                XX = sqh[:, 0:F]
                YY = sqh[:, F:2 * F]
                ZZ = sqh[:, 2 * F:3 * F]

                xy = pmono.tile([128, F], f32, tag="xy")
                nc.vector.tensor_mul(xy[:], X, Y)
                t8 = pmono.tile([128, F], f32, tag="t8")
                nc.vector.tensor_sub(t8[:], XX, YY)
                u9 = pmono.tile([128, F], f32, tag="u9")
                nc.vector.affine_then_add(u9[:], XX, t8[:], 2.0, 0.0)
                u15 = pmono.tile([128, F], f32, tag="u15")
                nc.vector.affine_then_add(u15[:], YY, t8[:], -2.0, 0.0)

                # ---- MAC: pair-batched products ([k2, c, f] = 4F per op) ----
                # Basis planes are packed in pairs matching consecutive k so
                # each product instruction covers 2 k's x 2 channels, and the
                # add tree runs on [4F] chunks (terms halve each level).
                def bpair_tile():
                    return pb.tile([128, 2 * F], bf16, tag="b", name="bp")

                def mk_product2(p_idx, bp):
                    m = pm.tile([128, 4 * F], bf16, tag="m", name="m")
                    in1 = shin[:, 4 * p_idx * F:(4 * p_idx + 4) * F].rearrange(
                        "p (k c f) -> p k c f", k=2, c=2)
                    in0 = bp[:].rearrange("p (k f) -> p k f", k=2) \
                        .unsqueeze(2).broadcast_to((128, 2, 2, F))
                    nc.vector.tensor_tensor(
                        m[:].rearrange("p (k c f) -> p k c f", k=2, c=2),
                        in0, in1, OP.mult)
                    return m

                def amr(out_slice, in0, in1, scale, bias):
                    scr = pscr.tile([128, 1], f32, tag="scr", name="scr")
                    nc.vector.affine_mul_reduce(out_slice, scr[:], in0, in1,
                                                scale, bias)

                def tadd(a, b_, dt):
                    tg = "treeb" if dt == bf16 else "treef"
                    nb = 4
                    o = ptree.tile([128, a.shape[1]], dt, tag=tg, name="tr",
                                   bufs=nb)
                    nc.vector.tensor_tensor(o[:], a[:], b_[:], OP.add)
                    return o

                # pair (k0, k1): [C0 const, -Ytilde]
                bp0 = bpair_tile()
                nc.vector.memset(bp0[:, 0:F], C0)
                nc.scalar.mul(bp0[:, F:2 * F], Y, -1.0)
                m0 = mk_product2(0, bp0)
                # pair (k2, k3): [Ztilde, -Xtilde]
                bp1 = bpair_tile()
                nc.scalar.copy(bp1[:, 0:F], Z)
                nc.scalar.mul(bp1[:, F:2 * F], X, -1.0)
                m1 = mk_product2(1, bp1)
                A = tadd(m0, m1, bf16)

                # pair (k4, k5): [c4*xy, c5*Y*Z]
                bp2 = bpair_tile()
                nc.scalar.mul(bp2[:, 0:F], xy[:], CC4)
                nc.vector.scalar_tensor_tensor(bp2[:, F:2 * F], Y, CC5, Z,
                                               OP.mult, OP.mult)
                m2 = mk_product2(2, bp2)
                # pair (k6, k7): [a6*ZZ+d6, c7*X*Z]
                bp3 = bpair_tile()
                nc.scalar.activation(bp3[:, 0:F], ZZ, AF.Identity, bias=ct[:, 3:4], scale=A6)
                nc.vector.scalar_tensor_tensor(bp3[:, F:2 * F], X, CC7, Z,
                                               OP.mult, OP.mult)
                m3 = mk_product2(3, bp3)
                B = tadd(m2, m3, bf16)

                # pair (k8, k9): [c8*t8, c9*u9*Y]
                bp4 = bpair_tile()
                nc.scalar.mul(bp4[:, 0:F], t8[:], CC8)
                amr(bp4[:, F:2 * F], u9[:], Y, CC9, 0.0)
                m4 = mk_product2(4, bp4)
                # pair (k10, k11): [c10*xy*Z, (a11*ZZ+d11)*Y]
                bp5 = bpair_tile()
                amr(bp5[:, 0:F], xy[:], Z, CC10, 0.0)
                amr(bp5[:, F:2 * F], ZZ, Y, A11, D11)
                m5 = mk_product2(5, bp5)
                Cc = tadd(m4, m5, bf16)

                # pair (k12, k13)
                bp6 = bpair_tile()
                amr(bp6[:, 0:F], ZZ, Z, A12, D12)
                amr(bp6[:, F:2 * F], ZZ, X, A13, D13)
                m6 = mk_product2(6, bp6)
                # pair (k14, k15)
                bp7 = bpair_tile()
                amr(bp7[:, 0:F], t8[:], Z, CC14, 0.0)
                amr(bp7[:, F:2 * F], u15[:], X, CC15, 0.0)
                m7 = mk_product2(7, bp7)
                D = tadd(m6, m7, bf16)

                E = tadd(A, B, f32)
                G = tadd(Cc, D, f32)
                H = tadd(E, G, f32)
                acc = pacc.tile([128, 2 * F], f32, tag="acc")
                nc.vector.tensor_add(acc[:], H[:, 0:2 * F], H[:, 2 * F:4 * F])

                # interleave back to (f, c) and upcast to fp32
                out_t = pout.tile([128, 2 * F], f32, tag="out")
                nc.scalar.copy(
                    out_t[:].rearrange("p (f c) -> p c f", c=2),
                    acc[:].rearrange("p (c f) -> p c f", c=2),
                )
                nc.sync.dma_start(
                    out=out_ap[:, t * 2 * F:(t + 1) * 2 * F], in_=out_t[:]
                )

    nc.finalize()
    return nc


_NC_CACHE = None
_last_in_maps = None


def _get_nc():
    global _NC_CACHE
    if _NC_CACHE is None:
        _NC_CACHE = _build_nc()
    return _NC_CACHE


def kernel(coordinates, active_deg, max_coeffs, sh_coefficients, rx_pos,
           **unused):
    assert int(active_deg) == ACTIVE_DEG and int(max_coeffs) == K
    coords = np.ascontiguousarray(np.asarray(coordinates, dtype=np.float32))
    sh = np.ascontiguousarray(np.asarray(sh_coefficients, dtype=np.float32))
    rx = np.asarray(rx_pos, dtype=np.float32).reshape(3)
    n = coords.shape[0]
    assert n == N and sh.shape == (N * K, CH)

    consts = np.zeros((128, 4), dtype=np.float32)
    consts[:, 0:3] = -rx[None, :]
    consts[:, 3] = D6

    # Host-side relayout: coords -> 3 fp32 planes, sh -> 32 bf16 (k,c)-planes,
    # so device DMAs land directly in compute layout.
    sh32 = sh.reshape(n, K * CH)
    coordsT = coords.T  # [3, N] view
    in_maps = []
    for c in range(NCORES):
        lo, hi = c * PC, (c + 1) * PC
        real = min(hi, n) - lo
        coords_c = np.zeros((3, PC), dtype=np.float32)
        coords_c[:, :real] = coordsT[:, lo:lo + real]
        sh_c = np.zeros((32, PC), dtype=ml_dtypes.bfloat16)
        sh_c[:, :real] = sh32[lo:lo + real].T
        in_maps.append({"coords": coords_c, "sh": sh_c, "consts": consts})

    global _last_in_maps
    _last_in_maps = in_maps
    res = run_bass_kernel_spmd(_get_nc(), in_maps, list(range(NCORES)))
    out = np.concatenate([np.asarray(res.results[c]["out"])
                          for c in range(NCORES)], axis=0)
    return out[:n]
